# revision 21
# baseline (speedup 1.0000x reference)
"""Trainium2 Bass kernel for nn_Attention_23218593202595.

reference:
    hidden = concat([static, dynamic, broadcast(decoder)], axis=1)   # [B, 3H, S]
    u      = tanh(einsum('hk,bks->bhs', W[0], hidden))               # [B, H, S]
    scores = einsum('h,bhs->bs', v[0,0], u)[:, None, :]              # [B, 1, S]
    out    = softmax(scores, axis=2)

B=256, H=256, S=512.  Pure data parallel over 8 NeuronCores: core i owns
batches [32i, 32i+32).  W/v/decoder-projection are tiny and replicated.

Per core, per batch b:
    psum_u[mc]  = sum_kc Wt[kc, mc]^T @ x[kc]       (x = [static;dynamic], bf16)
    u[mc]       = tanh(psum_u[mc] + c[:, b])        (ScalarE, c = W_dec @ dec)
    score chunk: for each 128-wide s-chunk, the u-chunk is loaded as the
    STATIONARY operand and v streams as a 1-column moving operand:
        sps[p, b%4, c] += u[mc][:, 128c:128c+128]^T @ vm[:, mc]   (N=1 matmul)
    so the score matmuls cost ~8x60 PE cycles/batch instead of 2x512-column
    streams.  Scores land s-on-partitions: sps[p, q, c] = score[c*128+p].
Softmax per 4-batch group in that layout:
    praw = exp(sps)                 (ScalarE, no max-subtraction: |score|<~10)
    sums[p, q]   = reduce_c praw    (DVE segmented reduce)
    totals[p, q] = ones^T @ sums    (one N=4 matmul: cross-partition sum,
                                     result broadcast to all 128 partitions)
    out = praw * reciprocal(totals) (DVE), DMAed as [128, 16] blocks; the
    host un-permutes [p, q, c] -> [b, c*128+p] after gather.

All inputs are converted to bf16 and pre-swizzled on the host into
partition-major layouts so every DMA lands as 128 contiguous runs.
"""
import sys

if "/opt/trn_rl_repo" not in sys.path:
    sys.path.insert(0, "/opt/trn_rl_repo")

import numpy as np
import ml_dtypes

B, H, S = 256, 256, 512
N_CORES = 8
NB = B // N_CORES                    # batches per core
GROUPS = [1, 1] + [2] * 15           # batches per dma group (sum = NB);
                                     # two singles up front so batch 0/1 don't
                                     # gate on their pair-mate's bytes; pairs
                                     # after that (single 524KB transfers pay
                                     # a per-transfer ramp that hurts the
                                     # sustained DMA rate)

_cache = {}


def _build():
    import concourse.bass as bass
    import concourse.mybir as mybir
    import concourse.tile as tile
    from concourse import bacc

    f32 = mybir.dt.float32
    bf16 = mybir.dt.bfloat16
    AF = mybir.ActivationFunctionType

    nc = bacc.Bacc("TRN2", target_bir_lowering=False, debug=False,
                   num_devices=N_CORES)

    # all host-preswizzled, partition-major.  xin interleaves static+dynamic
    # per batch as [b, tensor, c, s] so one DMA covers a whole batch group;
    # consts packs wt | dect | vm into a single transfer.
    CW = 6 * H + 2 * NB + 2
    xin_ext = nc.declare_dram_parameter("xin", [128, NB * 4 * S], bf16, isOutput=False)
    cst_ext = nc.declare_dram_parameter("consts", [128, CW], bf16, isOutput=False)
    out_ext = nc.declare_dram_parameter("out", [128, NB * 4], f32, isOutput=True)

    with tile.TileContext(nc) as tc:
        with (
            tc.tile_pool(name="const", bufs=1) as cpool,
            tc.tile_pool(name="inp", bufs=8) as ipool,
            tc.tile_pool(name="upool", bufs=6) as upool,
            tc.tile_pool(name="wpool", bufs=12) as wpool,
            tc.tile_pool(name="smpool", bufs=3) as smpool,
            tc.tile_pool(name="ps_u", bufs=5, space=bass.MemorySpace.PSUM) as ps_u,
            tc.tile_pool(name="ps_s", bufs=3, space=bass.MemorySpace.PSUM) as ps_s,
        ):
            # ---- constants: head/tail split on the sync queue ----
            # host layout: [kc4, kc5, de, vm, kc0, kc1 | kc2, kc3].  The
            # head (everything the c-matmuls and batch-0 static matmuls
            # need) goes first; the tail (kc2,kc3 — first needed ~0.9us
            # into batch 0) is issued after x0 so x0's transfer starts
            # ~1us earlier.  Tile's byte-range deps keep this sound.
            HEAD = 2 * H + 2 * NB + 2 + 2 * H   # kc4,kc5,de,vm,kc0,kc1
            cst_sb = cpool.tile([128, CW], bf16)
            nc.sync.dma_start(out=cst_sb[:, :HEAD], in_=cst_ext.ap()[:, :HEAD])
            WOFF = {4: 0, 5: H, 0: 2 * H + 2 * NB + 2,
                    1: 3 * H + 2 * NB + 2, 2: HEAD, 3: HEAD + H}

            def wt_c(kc, mc):
                o = WOFF[kc] + mc * 128
                return cst_sb[:, o:o + 128]

            de_sb = cst_sb[:, 2 * H:2 * H + 2 * NB] \
                .rearrange("p (c b) -> p c b", b=NB)           # [p, c, b]
            vm_sb = cst_sb[:, 2 * H + 2 * NB:2 * H + 2 * NB + 2]  # [p, mc]
            # f32 copy of v: tensor_scalar/scalar_tensor_tensor need an
            # f32 per-partition scalar operand
            vmf = cpool.tile([128, 2], f32)
            nc.vector.tensor_copy(vmf[:], vm_sb[:])

            # ones stationary for the cross-partition sum+broadcast matmul.
            # bf16: an fp32 PE matmul lowers to a LOW/HIGH double pass
            # (~390ns); bf16 keeps it at the ~25ns small-matmul floor, so
            # sums are down-converted to bf16 first (exact: ones are 1.0,
            # the f32->bf16 rounding is ~0.2%/sqrt(128) on the total).
            ones_sb = cpool.tile([128, 128], bf16)
            nc.gpsimd.memset(ones_sb[:], 1.0)

            # PE warmup while DMAs land: dense N=512 matmuls on a zeroed
            # tile so the HAM clock gate opens before the real stream starts
            # (also lets the input DMA build a 2-3 batch buffer; without it
            # the stream hits x-wait gaps around batch 5 — measured)
            warm = cpool.tile([128, S], bf16)
            nc.gpsimd.memset(warm[:], 0.0)
            wpsum = ps_s.tile([64, S], f32, tag="sps")
            # 12 matmuls: the PE consumes batches at ~1.83us vs the DMA's
            # ~1.87us delivery — main matmuls must start far enough behind
            # the DMA that the PE never catches it mid-stream
            for _ in range(12):
                nc.tensor.matmul(wpsum[:], warm[:, :64], warm[:])

            cbias = cpool.tile([128, 2, NB], f32)

            def emit_cmms():
                # c = W_dec @ dec^T  -> [H, NB] f32, kept as tanh bias
                for mc in range(2):
                    pc = ps_s.tile([128, NB], f32, tag="sps")
                    for kc in range(2):
                        nc.tensor.matmul(
                            pc[:],
                            wt_c(4 + kc, mc),
                            de_sb[:, kc, :],
                            start=(kc == 0), stop=(kc == 1),
                        )
                    nc.vector.tensor_copy(cbias[:, mc, :], pc[:])

            def score_mms(sps, q, wc):
                # wc = v0*u0 + v1*u1 (v folded on the DVE), so one N=1
                # matmul per 128-wide s-chunk with a ones moving column
                # does the 128-deep cross-partition sum: 4 tiny matmuls
                # per batch, no psum accumulation pairing needed.
                for c in range(4):
                    nc.tensor.matmul(
                        sps[:, q, c:c + 1],
                        wc[:, 128 * c:128 * (c + 1)],
                        ones_sb[:, 0:1],
                        start=True, stop=True,
                    )

            def stage1_finish(praw):
                # per-batch exps are emitted inside emit_group_scores (each
                # fires as soon as that batch's 8 score matmuls stop); here
                # just the partial-sum reduce + bf16 downcast for the
                # ones-matmul moving operand.
                sums = smpool.tile([128, 4], f32, tag="sm")
                sums_bf = smpool.tile([128, 4], bf16, tag="sb")
                nc.vector.tensor_reduce(
                    sums[:], praw[:],
                    axis=mybir.AxisListType.X, op=mybir.AluOpType.add)
                nc.vector.tensor_copy(sums_bf[:], sums[:])
                return sums_bf

            def stage2(g, praw, sums, last):
                # cross-partition sum of sums, broadcast to all partitions
                tot = ps_s.tile([128, 4], f32, tag="sps")
                nc.tensor.matmul(tot[:], ones_sb[:], sums[:],
                                 start=True, stop=True)
                recip = smpool.tile([128, 4], f32, tag="rc")
                nc.vector.reciprocal(recip[:], tot[:])
                outp = smpool.tile([128, 4, 4], f32, tag="op")
                for q in range(4):
                    nc.vector.tensor_scalar_mul(
                        outp[:, q, :], praw[:, q, :], recip[:, q:q + 1])
                # early groups ride the idle gpsimd SWDGE so the sync queue
                # keeps feeding input; the last group stays on sync (SWDGE's
                # ~2us completion latency would land in the kernel tail)
                dma_eng = nc.sync if last else nc.gpsimd
                dma_eng.dma_start(
                    out=out_ext.ap()[:, 16 * g:16 * (g + 1)],
                    in_=outp[:])

            # ---- main loop over batches ----
            pending = []   # batches whose scores are not yet emitted
            sm2 = []       # groups awaiting stage2
            b0 = 0

            # c-matmuls need only consts (which lands first) — run them
            # before batch 0 so they fill the consts->x0 arrival gap
            emit_cmms()

            def emit_group_scores():
                g = pending[0][0] // 4
                sps = ps_s.tile([128, 4, 4], f32, tag="sps")
                praw = smpool.tile([128, 4, 4], f32, tag="pr")
                for q, (b, wc) in enumerate(pending[:4]):
                    score_mms(sps, q, wc)
                # exps after all bursts in program order (they still run
                # eagerly on ScalarE as each batch's matmuls stop); emitting
                # one between bursts serializes the tail's psum-bank deps.
                # no max-subtraction: |score| <= ||v||_1 ~ 10, exp stays
                # comfortably inside f32 range.
                for q in range(4):
                    nc.scalar.activation(praw[:, q, :], sps[:, q, :], AF.Exp)
                del pending[:4]
                sums = stage1_finish(praw)
                sm2.append((g, praw, sums))

            for gi, gsz in enumerate(GROUPS):
                x_t = ipool.tile([128, gsz, 2, 2, S], bf16, tag="xt")
                nc.sync.dma_start(
                    out=x_t[:],
                    in_=xin_ext.ap()[:, b0 * 4 * S:(b0 + gsz) * 4 * S])
                if gi == 0:
                    nc.sync.dma_start(out=cst_sb[:, HEAD:],
                                      in_=cst_ext.ap()[:, HEAD:])

                for j in range(gsz):
                    b = b0 + j
                    psu_list = []
                    for mc in range(2):
                        psu = ps_u.tile([128, S], f32, tag="ups")
                        for kc in range(2):
                            nc.tensor.matmul(
                                psu[:],
                                wt_c(kc, mc),
                                x_t[:, j, 0, kc, :],
                                start=(kc == 0), stop=False,
                            )
                        for kc in range(2):
                            nc.tensor.matmul(
                                psu[:],
                                wt_c(2 + kc, mc),
                                x_t[:, j, 1, kc, :],
                                start=False, stop=(kc == 1),
                            )
                        psu_list.append(psu)
                    u_list = []
                    for mc in range(2):
                        u_bf = upool.tile([128, S], bf16, tag="u")
                        nc.scalar.activation(u_bf[:], psu_list[mc][:], AF.Tanh,
                                             bias=cbias[:, mc, b:b + 1])
                        u_list.append(u_bf)
                    # fold v into u on the DVE: wc = v0*u0 + v1*u1
                    w0 = wpool.tile([128, S], bf16, tag="w0")
                    wc = wpool.tile([128, S], bf16, tag="wc")
                    nc.vector.tensor_scalar_mul(w0[:], u_list[0][:],
                                                vmf[:, 0:1])
                    nc.vector.scalar_tensor_tensor(
                        wc[:], u_list[1][:], vmf[:, 1:2], w0[:],
                        op0=mybir.AluOpType.mult, op1=mybir.AluOpType.add)
                    # group score matmuls run 1+ batch behind the mains;
                    # stage2 of a group runs one group later so its PE
                    # matmul never heads the queue before sums are ready
                    pending.append((b, wc))
                    if len(pending) >= 5:
                        emit_group_scores()
                        if len(sm2) > 1:
                            g, praw, sums = sm2.pop(0)
                            stage2(g, praw, sums, last=False)
                b0 += gsz
            while pending:
                emit_group_scores()
            while sm2:
                g, praw, sums = sm2.pop(0)
                stage2(g, praw, sums, last=(not sm2))

    nc.compile()
    return nc


def _get_nc():
    if "nc" not in _cache:
        _cache["nc"] = _build()
    return _cache["nc"]


def make_in_maps(static_hidden, dynamic_hidden, decoder_hidden, W, v):
    bf = ml_dtypes.bfloat16
    # W[0] is [H, 3H]; wt[p, kc*H + m] = W[0][m, kc*128 + p]
    wt = W[0].T.astype(bf).reshape(6, 128, H).transpose(1, 0, 2).reshape(128, 6 * H)
    # v as the moving operand: vm[p, mc] = v[mc*128 + p]
    vm = np.ascontiguousarray(v[0, 0].astype(bf).reshape(2, 128).T)

    sh = static_hidden.astype(bf).reshape(N_CORES, NB, 2, 128, S)
    dh = dynamic_hidden.astype(bf).reshape(N_CORES, NB, 2, 128, S)
    # xin[p, b, t, c, s]; h = c*128 + p
    xin_all = np.stack([sh, dh], axis=2).transpose(0, 4, 1, 2, 3, 5) \
        .reshape(N_CORES, 128, NB * 4 * S)

    in_maps = []
    for i in range(N_CORES):
        sl = slice(i * NB, (i + 1) * NB)
        dect = decoder_hidden[sl].T.astype(bf).reshape(2, 128, NB) \
            .transpose(1, 0, 2).reshape(128, 2 * NB)
        wtc = wt.reshape(128, 6, H)
        consts = np.concatenate(
            [wtc[:, 4], wtc[:, 5], dect, vm,
             wtc[:, 0], wtc[:, 1], wtc[:, 2], wtc[:, 3]], axis=1)
        in_maps.append({
            "xin": np.ascontiguousarray(xin_all[i]),
            "consts": np.ascontiguousarray(consts),
        })
    return in_maps


def kernel(static_hidden, dynamic_hidden, decoder_hidden, W, v):
    from concourse.bass_utils import run_bass_kernel_spmd

    static_hidden = np.asarray(static_hidden, dtype=np.float32)
    dynamic_hidden = np.asarray(dynamic_hidden, dtype=np.float32)
    decoder_hidden = np.asarray(decoder_hidden, dtype=np.float32)
    W = np.asarray(W, dtype=np.float32)
    v = np.asarray(v, dtype=np.float32)
    nc = _get_nc()
    in_maps = make_in_maps(static_hidden, dynamic_hidden, decoder_hidden, W, v)
    res = run_bass_kernel_spmd(nc, in_maps, list(range(N_CORES)))
    # device layout: out[p, 4b + c] = prob[b, c*128 + p]
    parts = []
    for i in range(N_CORES):
        r = res.results[i]["out"].reshape(128, NB, 4)
        parts.append(r.transpose(1, 2, 0).reshape(NB, S))
    out = np.concatenate(parts, axis=0)
    return out[:, None, :].astype(np.float32)


# revision 26
# speedup vs baseline: 1.0184x; 1.0184x over previous
"""Trainium2 Bass kernel for nn_Attention_23218593202595.

reference:
    hidden = concat([static, dynamic, broadcast(decoder)], axis=1)   # [B, 3H, S]
    u      = tanh(einsum('hk,bks->bhs', W[0], hidden))               # [B, H, S]
    scores = einsum('h,bhs->bs', v[0,0], u)[:, None, :]              # [B, 1, S]
    out    = softmax(scores, axis=2)

B=256, H=256, S=512.  Pure data parallel over 8 NeuronCores: core i owns
batches [32i, 32i+32).  W/v/decoder-projection are tiny and replicated.

Per core, per batch b:
    psum_u[mc]  = sum_kc Wt[kc, mc]^T @ x[kc]       (x = [static;dynamic], bf16)
    u[mc]       = tanh(psum_u[mc] + c[:, b])        (ScalarE, c = W_dec @ dec)
    score chunk: for each 128-wide s-chunk, the u-chunk is loaded as the
    STATIONARY operand and v streams as a 1-column moving operand:
        sps[p, b%4, c] += u[mc][:, 128c:128c+128]^T @ vm[:, mc]   (N=1 matmul)
    so the score matmuls cost ~8x60 PE cycles/batch instead of 2x512-column
    streams.  Scores land s-on-partitions: sps[p, q, c] = score[c*128+p].
Softmax per 4-batch group in that layout:
    praw = exp(sps)                 (ScalarE, no max-subtraction: |score|<~10)
    sums[p, q]   = reduce_c praw    (DVE segmented reduce)
    totals[p, q] = ones^T @ sums    (one N=4 matmul: cross-partition sum,
                                     result broadcast to all 128 partitions)
    out = praw * reciprocal(totals) (DVE), DMAed as [128, 16] blocks; the
    host un-permutes [p, q, c] -> [b, c*128+p] after gather.

All inputs are converted to bf16 and pre-swizzled on the host into
partition-major layouts so every DMA lands as 128 contiguous runs.
"""
import sys

if "/opt/trn_rl_repo" not in sys.path:
    sys.path.insert(0, "/opt/trn_rl_repo")

import numpy as np
import ml_dtypes

B, H, S = 256, 256, 512
N_CORES = 8
NB = B // N_CORES                    # batches per core
GROUPS = [1, 1] + [2] * 15           # batches per dma group (sum = NB);
                                     # two singles up front so batch 0/1 don't
                                     # gate on their pair-mate's bytes; pairs
                                     # after that (single 524KB transfers pay
                                     # a per-transfer ramp that hurts the
                                     # sustained DMA rate)

_cache = {}


def _build():
    import concourse.bass as bass
    import concourse.mybir as mybir
    import concourse.tile as tile
    from concourse import bacc

    f32 = mybir.dt.float32
    bf16 = mybir.dt.bfloat16
    AF = mybir.ActivationFunctionType

    nc = bacc.Bacc("TRN2", target_bir_lowering=False, debug=False,
                   num_devices=N_CORES)

    # all host-preswizzled, partition-major.  xin interleaves static+dynamic
    # per batch as [b, tensor, c, s] so one DMA covers a whole batch group;
    # consts packs wt | dect | vm into a single transfer.
    CW = 6 * H + 2 * NB + 2
    xin_ext = nc.declare_dram_parameter("xin", [128, NB * 4 * S], bf16, isOutput=False)
    cst_ext = nc.declare_dram_parameter("consts", [128, CW], bf16, isOutput=False)
    out_ext = nc.declare_dram_parameter("out", [128, NB * 4], f32, isOutput=True)

    with tile.TileContext(nc) as tc:
        with (
            tc.tile_pool(name="const", bufs=1) as cpool,
            tc.tile_pool(name="inp", bufs=8) as ipool,
            tc.tile_pool(name="upool", bufs=6) as upool,
            tc.tile_pool(name="wpool", bufs=12) as wpool,
            tc.tile_pool(name="smpool", bufs=3) as smpool,
            tc.tile_pool(name="ps_u", bufs=5, space=bass.MemorySpace.PSUM) as ps_u,
            tc.tile_pool(name="ps_s", bufs=3, space=bass.MemorySpace.PSUM) as ps_s,
        ):
            # ---- constants: head/tail split on the sync queue ----
            # host layout: [kc4, kc5, de, vm, kc0, kc1 | kc2, kc3].  The
            # head (everything the c-matmuls and batch-0 static matmuls
            # need) goes first; the tail (kc2,kc3 — first needed ~0.9us
            # into batch 0) is issued after x0 so x0's transfer starts
            # ~1us earlier.  Tile's byte-range deps keep this sound.
            HEAD = 2 * H + 2 * NB + 2 + 2 * H   # kc4,kc5,de,vm,kc0,kc1
            cst_sb = cpool.tile([128, CW], bf16)
            nc.sync.dma_start(out=cst_sb[:, :HEAD], in_=cst_ext.ap()[:, :HEAD])
            WOFF = {4: 0, 5: H, 0: 2 * H + 2 * NB + 2,
                    1: 3 * H + 2 * NB + 2, 2: HEAD, 3: HEAD + H}

            def wt_c(kc, mc):
                o = WOFF[kc] + mc * 128
                return cst_sb[:, o:o + 128]

            de_sb = cst_sb[:, 2 * H:2 * H + 2 * NB] \
                .rearrange("p (c b) -> p c b", b=NB)           # [p, c, b]
            vm_sb = cst_sb[:, 2 * H + 2 * NB:2 * H + 2 * NB + 2]  # [p, mc]
            # f32 copy of v: tensor_scalar/scalar_tensor_tensor need an
            # f32 per-partition scalar operand
            vmf = cpool.tile([128, 2], f32)
            nc.vector.tensor_copy(vmf[:], vm_sb[:])

            # ones stationary for the cross-partition sum+broadcast matmul.
            # bf16: an fp32 PE matmul lowers to a LOW/HIGH double pass
            # (~390ns); bf16 keeps it at the ~25ns small-matmul floor, so
            # sums are down-converted to bf16 first (exact: ones are 1.0,
            # the f32->bf16 rounding is ~0.2%/sqrt(128) on the total).
            ones_sb = cpool.tile([128, 128], bf16)
            nc.gpsimd.memset(ones_sb[:], 1.0)

            # PE warmup while DMAs land: dense N=512 matmuls on a zeroed
            # tile so the HAM clock gate opens before the real stream starts
            # (also lets the input DMA build a 2-3 batch buffer; without it
            # the stream hits x-wait gaps around batch 5 — measured)
            warm = cpool.tile([128, S], bf16)
            nc.gpsimd.memset(warm[:], 0.0)
            wpsum = ps_s.tile([64, S], f32, tag="sps")
            # 9 matmuls (~3.8us cold): bridges until x0 lands and opens the
            # HAM clock gate before the real stream starts
            for _ in range(9):
                nc.tensor.matmul(wpsum[:], warm[:, :64], warm[:])

            cbias = cpool.tile([128, 2, NB], f32)

            def emit_cmms():
                # c = W_dec @ dec^T  -> [H, NB] f32, kept as tanh bias
                for mc in range(2):
                    pc = ps_s.tile([128, NB], f32, tag="sps")
                    for kc in range(2):
                        nc.tensor.matmul(
                            pc[:],
                            wt_c(4 + kc, mc),
                            de_sb[:, kc, :],
                            start=(kc == 0), stop=(kc == 1),
                        )
                    nc.vector.tensor_copy(cbias[:, mc, :], pc[:])

            def score_mms(sps, q, wc):
                # wc = v0*u0 + v1*u1 (v folded on the DVE), so one N=1
                # matmul per 128-wide s-chunk with a ones moving column
                # does the 128-deep cross-partition sum: 4 tiny matmuls
                # per batch, no psum accumulation pairing needed.
                for c in range(4):
                    nc.tensor.matmul(
                        sps[:, q, c:c + 1],
                        wc[:, 128 * c:128 * (c + 1)],
                        ones_sb[:, 0:1],
                        start=True, stop=True,
                    )

            def score_mms_u(sps, q, u_list):
                # last-batch path: u-chunk stationary, v moving — no DVE op
                # in the dependency chain behind the final tanh.  The mc0
                # half only needs tanh(mc0), so those 4 matmuls run while
                # tanh(mc1) is still on the ScalarE.  Each column's
                # start/stop pair must be adjacent (start clears the whole
                # bank's has_written).
                for c in range(4):
                    for mc in range(2):
                        nc.tensor.matmul(
                            sps[:, q, c:c + 1],
                            u_list[mc][:, 128 * c:128 * (c + 1)],
                            vm_sb[:, mc:mc + 1],
                            start=(mc == 0), stop=(mc == 1),
                        )

            def stage1_finish(praw):
                # per-batch exps are emitted inside emit_group_scores (each
                # fires as soon as that batch's 8 score matmuls stop); here
                # just the partial-sum reduce + bf16 downcast for the
                # ones-matmul moving operand.
                sums = smpool.tile([128, 4], f32, tag="sm")
                sums_bf = smpool.tile([128, 4], bf16, tag="sb")
                nc.vector.tensor_reduce(
                    sums[:], praw[:],
                    axis=mybir.AxisListType.X, op=mybir.AluOpType.add)
                nc.vector.tensor_copy(sums_bf[:], sums[:])
                return sums_bf

            def stage2(g, praw, sums, last):
                # cross-partition sum of sums, broadcast to all partitions
                tot = ps_s.tile([128, 4], f32, tag="sps")
                nc.tensor.matmul(tot[:], ones_sb[:], sums[:],
                                 start=True, stop=True)
                recip = smpool.tile([128, 4], f32, tag="rc")
                nc.vector.reciprocal(recip[:], tot[:])
                outp = smpool.tile([128, 4, 4], f32, tag="op")
                for q in range(4):
                    nc.vector.tensor_scalar_mul(
                        outp[:, q, :], praw[:, q, :], recip[:, q:q + 1])
                # early groups ride the idle gpsimd SWDGE so the sync queue
                # keeps feeding input; the last group stays on sync (SWDGE's
                # ~2us completion latency would land in the kernel tail)
                dma_eng = nc.sync if last else nc.gpsimd
                dma_eng.dma_start(
                    out=out_ext.ap()[:, 16 * g:16 * (g + 1)],
                    in_=outp[:])

            # ---- main loop over batches ----
            pending = []   # batches whose scores are not yet emitted
            sm2 = []       # groups awaiting stage2
            gstate = {}    # group -> (sps, praw)
            b0 = 0

            # c-matmuls need only consts (which lands first) — run them
            # before batch 0 so they fill the consts->x0 arrival gap
            emit_cmms()

            def score_one():
                # per-batch: 4 tiny matmuls + this batch's exp; the group's
                # reduce fires after its 4th batch.  no max-subtraction:
                # |score| <= ||v||_1 ~ 10, exp stays comfortably inside
                # f32 range.
                b, wc, u_list = pending.pop(0)
                g, q = b // 4, b % 4
                if q == 0:
                    sps_t = ps_s.tile([128, 4, 4], f32, tag="sps")
                    praw_t = smpool.tile([128, 4, 4], f32, tag="pr")
                    gstate[g] = (sps_t, praw_t)
                sps, praw = gstate[g]
                if wc is None:
                    score_mms_u(sps, q, u_list)
                else:
                    score_mms(sps, q, wc)
                nc.scalar.activation(praw[:, q, :], sps[:, q, :], AF.Exp)
                if q == 3:
                    sums = stage1_finish(praw)
                    sm2.append((g, praw, sums))
                    del gstate[g]
                # stage2 one group behind: its PE matmul never heads the
                # queue before the DVE-produced sums are ready
                if len(sm2) > 1:
                    gg, praw2, sums2 = sm2.pop(0)
                    stage2(gg, praw2, sums2, last=False)

            for gi, gsz in enumerate(GROUPS):
                x_t = ipool.tile([128, gsz, 2, 2, S], bf16, tag="xt")
                nc.sync.dma_start(
                    out=x_t[:],
                    in_=xin_ext.ap()[:, b0 * 4 * S:(b0 + gsz) * 4 * S])
                if gi == 0:
                    nc.sync.dma_start(out=cst_sb[:, HEAD:],
                                      in_=cst_ext.ap()[:, HEAD:])

                for j in range(gsz):
                    b = b0 + j
                    psu_list = []
                    for mc in range(2):
                        psu = ps_u.tile([128, S], f32, tag="ups")
                        for kc in range(2):
                            nc.tensor.matmul(
                                psu[:],
                                wt_c(kc, mc),
                                x_t[:, j, 0, kc, :],
                                start=(kc == 0), stop=False,
                            )
                        for kc in range(2):
                            nc.tensor.matmul(
                                psu[:],
                                wt_c(2 + kc, mc),
                                x_t[:, j, 1, kc, :],
                                start=False, stop=(kc == 1),
                            )
                        psu_list.append(psu)
                    u_list = []
                    for mc in range(2):
                        u_bf = upool.tile([128, S], bf16, tag="u")
                        nc.scalar.activation(u_bf[:], psu_list[mc][:], AF.Tanh,
                                             bias=cbias[:, mc, b:b + 1])
                        u_list.append(u_bf)
                    if b < NB - 1:
                        # fold v into u on the DVE: wc = v0*u0 + v1*u1
                        w0 = wpool.tile([128, S], bf16, tag="w0")
                        wc = wpool.tile([128, S], bf16, tag="wc")
                        nc.vector.tensor_scalar_mul(w0[:], u_list[0][:],
                                                    vmf[:, 0:1])
                        nc.vector.scalar_tensor_tensor(
                            wc[:], u_list[1][:], vmf[:, 1:2], w0[:],
                            op0=mybir.AluOpType.mult, op1=mybir.AluOpType.add)
                    else:
                        # last batch: keep the DVE out of the tail's
                        # dependency chain
                        wc = None
                    # score matmuls run 2 batches behind the mains
                    pending.append((b, wc, u_list))
                    if len(pending) >= 3:
                        score_one()
                b0 += gsz
            while pending:
                score_one()
            while sm2:
                g, praw, sums = sm2.pop(0)
                stage2(g, praw, sums, last=(not sm2))

    nc.compile()
    return nc


def _get_nc():
    if "nc" not in _cache:
        _cache["nc"] = _build()
    return _cache["nc"]


def make_in_maps(static_hidden, dynamic_hidden, decoder_hidden, W, v):
    bf = ml_dtypes.bfloat16
    # W[0] is [H, 3H]; wt[p, kc*H + m] = W[0][m, kc*128 + p]
    wt = W[0].T.astype(bf).reshape(6, 128, H).transpose(1, 0, 2).reshape(128, 6 * H)
    # v as the moving operand: vm[p, mc] = v[mc*128 + p]
    vm = np.ascontiguousarray(v[0, 0].astype(bf).reshape(2, 128).T)

    sh = static_hidden.astype(bf).reshape(N_CORES, NB, 2, 128, S)
    dh = dynamic_hidden.astype(bf).reshape(N_CORES, NB, 2, 128, S)
    # xin[p, b, t, c, s]; h = c*128 + p
    xin_all = np.stack([sh, dh], axis=2).transpose(0, 4, 1, 2, 3, 5) \
        .reshape(N_CORES, 128, NB * 4 * S)

    in_maps = []
    for i in range(N_CORES):
        sl = slice(i * NB, (i + 1) * NB)
        dect = decoder_hidden[sl].T.astype(bf).reshape(2, 128, NB) \
            .transpose(1, 0, 2).reshape(128, 2 * NB)
        wtc = wt.reshape(128, 6, H)
        consts = np.concatenate(
            [wtc[:, 4], wtc[:, 5], dect, vm,
             wtc[:, 0], wtc[:, 1], wtc[:, 2], wtc[:, 3]], axis=1)
        in_maps.append({
            "xin": np.ascontiguousarray(xin_all[i]),
            "consts": np.ascontiguousarray(consts),
        })
    return in_maps


def kernel(static_hidden, dynamic_hidden, decoder_hidden, W, v):
    from concourse.bass_utils import run_bass_kernel_spmd

    static_hidden = np.asarray(static_hidden, dtype=np.float32)
    dynamic_hidden = np.asarray(dynamic_hidden, dtype=np.float32)
    decoder_hidden = np.asarray(decoder_hidden, dtype=np.float32)
    W = np.asarray(W, dtype=np.float32)
    v = np.asarray(v, dtype=np.float32)
    nc = _get_nc()
    in_maps = make_in_maps(static_hidden, dynamic_hidden, decoder_hidden, W, v)
    res = run_bass_kernel_spmd(nc, in_maps, list(range(N_CORES)))
    # device layout: out[p, 4b + c] = prob[b, c*128 + p]
    parts = []
    for i in range(N_CORES):
        r = res.results[i]["out"].reshape(128, NB, 4)
        parts.append(r.transpose(1, 2, 0).reshape(NB, S))
    out = np.concatenate(parts, axis=0)
    return out[:, None, :].astype(np.float32)


# revision 28
# speedup vs baseline: 1.0253x; 1.0068x over previous
"""Trainium2 Bass kernel for nn_Attention_23218593202595.

reference:
    hidden = concat([static, dynamic, broadcast(decoder)], axis=1)   # [B, 3H, S]
    u      = tanh(einsum('hk,bks->bhs', W[0], hidden))               # [B, H, S]
    scores = einsum('h,bhs->bs', v[0,0], u)[:, None, :]              # [B, 1, S]
    out    = softmax(scores, axis=2)

B=256, H=256, S=512.  Pure data parallel over 8 NeuronCores: core i owns
batches [32i, 32i+32).  W/v/decoder-projection are tiny and replicated.

Per core, per batch b:
    psum_u[mc]  = sum_kc Wt[kc, mc]^T @ x[kc]       (x = [static;dynamic], bf16)
    u[mc]       = tanh(psum_u[mc] + c[:, b])        (ScalarE, c = W_dec @ dec)
    score chunk: for each 128-wide s-chunk, the u-chunk is loaded as the
    STATIONARY operand and v streams as a 1-column moving operand:
        sps[p, b%4, c] += u[mc][:, 128c:128c+128]^T @ vm[:, mc]   (N=1 matmul)
    so the score matmuls cost ~8x60 PE cycles/batch instead of 2x512-column
    streams.  Scores land s-on-partitions: sps[p, q, c] = score[c*128+p].
Softmax per 4-batch group in that layout:
    praw = exp(sps)                 (ScalarE, no max-subtraction: |score|<~10)
    sums[p, q]   = reduce_c praw    (DVE segmented reduce)
    totals[p, q] = ones^T @ sums    (one N=4 matmul: cross-partition sum,
                                     result broadcast to all 128 partitions)
    out = praw * reciprocal(totals) (DVE), DMAed as [128, 16] blocks; the
    host un-permutes [p, q, c] -> [b, c*128+p] after gather.

All inputs are converted to bf16 and pre-swizzled on the host into
partition-major layouts so every DMA lands as 128 contiguous runs.
"""
import sys

if "/opt/trn_rl_repo" not in sys.path:
    sys.path.insert(0, "/opt/trn_rl_repo")

import numpy as np
import ml_dtypes

B, H, S = 256, 256, 512
N_CORES = 8
NB = B // N_CORES                    # batches per core
GROUPS = [1, 1] + [2] * 15           # batches per dma group (sum = NB);
                                     # two singles up front so batch 0/1 don't
                                     # gate on their pair-mate's bytes; pairs
                                     # after that (single 524KB transfers pay
                                     # a per-transfer ramp that hurts the
                                     # sustained DMA rate)

_cache = {}


def _build():
    import concourse.bass as bass
    import concourse.mybir as mybir
    import concourse.tile as tile
    from concourse import bacc

    f32 = mybir.dt.float32
    bf16 = mybir.dt.bfloat16
    AF = mybir.ActivationFunctionType

    nc = bacc.Bacc("TRN2", target_bir_lowering=False, debug=False,
                   num_devices=N_CORES)

    # all host-preswizzled, partition-major.  xin interleaves static+dynamic
    # per batch as [b, tensor, c, s] so one DMA covers a whole batch group;
    # consts packs wt | dect | vm into a single transfer.
    CW = 6 * H + 2 * NB + 2
    xin_ext = nc.declare_dram_parameter("xin", [128, NB * 4 * S], bf16, isOutput=False)
    cst_ext = nc.declare_dram_parameter("consts", [128, CW], bf16, isOutput=False)
    out_ext = nc.declare_dram_parameter("out", [128, NB * 4], f32, isOutput=True)

    with tile.TileContext(nc) as tc:
        with (
            tc.tile_pool(name="const", bufs=1) as cpool,
            tc.tile_pool(name="inp", bufs=8) as ipool,
            tc.tile_pool(name="upool", bufs=6) as upool,
            tc.tile_pool(name="wpool", bufs=12) as wpool,
            tc.tile_pool(name="smpool", bufs=3) as smpool,
            tc.tile_pool(name="ps_u", bufs=5, space=bass.MemorySpace.PSUM) as ps_u,
            tc.tile_pool(name="ps_s", bufs=3, space=bass.MemorySpace.PSUM) as ps_s,
        ):
            # ---- constants: head/tail split on the sync queue ----
            # host layout: [kc4, kc5, de, vm, kc0, kc1 | kc2, kc3].  The
            # head (everything the c-matmuls and batch-0 static matmuls
            # need) goes first; the tail (kc2,kc3 — first needed ~0.9us
            # into batch 0) is issued after x0 so x0's transfer starts
            # ~1us earlier.  Tile's byte-range deps keep this sound.
            HEAD = 2 * H + 2 * NB + 2 + 2 * H   # kc4,kc5,de,vm,kc0,kc1
            cst_sb = cpool.tile([128, CW], bf16)
            nc.sync.dma_start(out=cst_sb[:, :HEAD], in_=cst_ext.ap()[:, :HEAD])
            WOFF = {4: 0, 5: H, 0: 2 * H + 2 * NB + 2,
                    1: 3 * H + 2 * NB + 2, 2: HEAD, 3: HEAD + H}

            def wt_c(kc, mc):
                o = WOFF[kc] + mc * 128
                return cst_sb[:, o:o + 128]

            de_sb = cst_sb[:, 2 * H:2 * H + 2 * NB] \
                .rearrange("p (c b) -> p c b", b=NB)           # [p, c, b]
            vm_sb = cst_sb[:, 2 * H + 2 * NB:2 * H + 2 * NB + 2]  # [p, mc]
            # f32 copy of v: tensor_scalar/scalar_tensor_tensor need an
            # f32 per-partition scalar operand
            vmf = cpool.tile([128, 2], f32)
            nc.vector.tensor_copy(vmf[:], vm_sb[:])

            # ones stationary for the cross-partition sum+broadcast matmul.
            # bf16: an fp32 PE matmul lowers to a LOW/HIGH double pass
            # (~390ns); bf16 keeps it at the ~25ns small-matmul floor, so
            # sums are down-converted to bf16 first (exact: ones are 1.0,
            # the f32->bf16 rounding is ~0.2%/sqrt(128) on the total).
            ones_sb = cpool.tile([128, 128], bf16)
            nc.gpsimd.memset(ones_sb[:], 1.0)

            # PE warmup while DMAs land: dense N=512 matmuls on a zeroed
            # tile so the HAM clock gate opens before the real stream starts
            # (also lets the input DMA build a 2-3 batch buffer; without it
            # the stream hits x-wait gaps around batch 5 — measured)
            warm = cpool.tile([128, S], bf16)
            nc.gpsimd.memset(warm[:], 0.0)
            wpsum = ps_s.tile([64, S], f32, tag="sps")
            # 11 matmuls (~4.2us, mostly at cold clock): bridges until x0/x1
            # land and opens the HAM clock gate before the real stream starts
            for _ in range(11):
                nc.tensor.matmul(wpsum[:], warm[:, :64], warm[:])

            cbias = cpool.tile([128, 2, NB], f32)

            def emit_cmms():
                # c = W_dec @ dec^T  -> [H, NB] f32, kept as tanh bias
                for mc in range(2):
                    pc = ps_s.tile([128, NB], f32, tag="sps")
                    for kc in range(2):
                        nc.tensor.matmul(
                            pc[:],
                            wt_c(4 + kc, mc),
                            de_sb[:, kc, :],
                            start=(kc == 0), stop=(kc == 1),
                        )
                    nc.vector.tensor_copy(cbias[:, mc, :], pc[:])

            def score_mms(sps, q, wc):
                # wc = v0*u0 + v1*u1 (v folded on the DVE), so one N=1
                # matmul per 128-wide s-chunk with a ones moving column
                # does the 128-deep cross-partition sum: 4 tiny matmuls
                # per batch, no psum accumulation pairing needed.
                for c in range(4):
                    nc.tensor.matmul(
                        sps[:, q, c:c + 1],
                        wc[:, 128 * c:128 * (c + 1)],
                        ones_sb[:, 0:1],
                        start=True, stop=True,
                    )

            def score_mms_u(sps, q, u_list):
                # last-batch path: u-chunk stationary, v moving — no DVE op
                # in the dependency chain behind the final tanh.  The mc0
                # half only needs tanh(mc0), so those 4 matmuls run while
                # tanh(mc1) is still on the ScalarE.  Each column's
                # start/stop pair must be adjacent (start clears the whole
                # bank's has_written).
                for c in range(4):
                    for mc in range(2):
                        nc.tensor.matmul(
                            sps[:, q, c:c + 1],
                            u_list[mc][:, 128 * c:128 * (c + 1)],
                            vm_sb[:, mc:mc + 1],
                            start=(mc == 0), stop=(mc == 1),
                        )

            def stage1_finish(praw):
                # per-batch exps are emitted inside emit_group_scores (each
                # fires as soon as that batch's 8 score matmuls stop); here
                # just the partial-sum reduce + bf16 downcast for the
                # ones-matmul moving operand.
                sums = smpool.tile([128, 4], f32, tag="sm")
                sums_bf = smpool.tile([128, 4], bf16, tag="sb")
                nc.vector.tensor_reduce(
                    sums[:], praw[:],
                    axis=mybir.AxisListType.X, op=mybir.AluOpType.add)
                nc.vector.tensor_copy(sums_bf[:], sums[:])
                return sums_bf

            def stage2(g, praw, sums, last):
                # cross-partition sum of sums, broadcast to all partitions
                tot = ps_s.tile([128, 4], f32, tag="sps")
                nc.tensor.matmul(tot[:], ones_sb[:], sums[:],
                                 start=True, stop=True)
                recip = smpool.tile([128, 4], f32, tag="rc")
                nc.vector.reciprocal(recip[:], tot[:])
                outp = smpool.tile([128, 4, 4], f32, tag="op")
                for q in range(4):
                    nc.vector.tensor_scalar_mul(
                        outp[:, q, :], praw[:, q, :], recip[:, q:q + 1])
                # early groups ride the idle gpsimd SWDGE so the sync queue
                # keeps feeding input; the last group stays on sync (SWDGE's
                # ~2us completion latency would land in the kernel tail)
                dma_eng = nc.sync if last else nc.gpsimd
                dma_eng.dma_start(
                    out=out_ext.ap()[:, 16 * g:16 * (g + 1)],
                    in_=outp[:])

            # ---- main loop over batches ----
            pending = []   # batches whose scores are not yet emitted
            sm2 = []       # groups awaiting stage2
            gstate = {}    # group -> (sps, praw)
            b0 = 0

            # c-matmuls need only consts (which lands first) — run them
            # before batch 0 so they fill the consts->x0 arrival gap
            emit_cmms()

            def score_one():
                # per-batch: 4 tiny matmuls + this batch's exp; the group's
                # reduce fires after its 4th batch.  no max-subtraction:
                # |score| <= ||v||_1 ~ 10, exp stays comfortably inside
                # f32 range.
                b, wc, u_list = pending.pop(0)
                g, q = b // 4, b % 4
                if q == 0:
                    sps_t = ps_s.tile([128, 4, 4], f32, tag="sps")
                    praw_t = smpool.tile([128, 4, 4], f32, tag="pr")
                    gstate[g] = (sps_t, praw_t)
                sps, praw = gstate[g]
                if wc is None:
                    score_mms_u(sps, q, u_list)
                else:
                    score_mms(sps, q, wc)
                nc.scalar.activation(praw[:, q, :], sps[:, q, :], AF.Exp)
                if q == 3:
                    sums = stage1_finish(praw)
                    sm2.append((g, praw, sums))
                    del gstate[g]
                # stage2 one group behind: its PE matmul never heads the
                # queue before the DVE-produced sums are ready
                if len(sm2) > 1:
                    gg, praw2, sums2 = sm2.pop(0)
                    stage2(gg, praw2, sums2, last=False)

            for gi, gsz in enumerate(GROUPS):
                x_t = ipool.tile([128, gsz, 2, 2, S], bf16, tag="xt")
                nc.sync.dma_start(
                    out=x_t[:],
                    in_=xin_ext.ap()[:, b0 * 4 * S:(b0 + gsz) * 4 * S])
                if gi == 0:
                    nc.sync.dma_start(out=cst_sb[:, HEAD:],
                                      in_=cst_ext.ap()[:, HEAD:])

                for j in range(gsz):
                    b = b0 + j
                    psu_list = []
                    for mc in range(2):
                        psu = ps_u.tile([128, S], f32, tag="ups")
                        for kc in range(2):
                            nc.tensor.matmul(
                                psu[:],
                                wt_c(kc, mc),
                                x_t[:, j, 0, kc, :],
                                start=(kc == 0), stop=False,
                            )
                        for kc in range(2):
                            nc.tensor.matmul(
                                psu[:],
                                wt_c(2 + kc, mc),
                                x_t[:, j, 1, kc, :],
                                start=False, stop=(kc == 1),
                            )
                        psu_list.append(psu)
                    u_list = []
                    for mc in range(2):
                        u_bf = upool.tile([128, S], bf16, tag="u")
                        nc.scalar.activation(u_bf[:], psu_list[mc][:], AF.Tanh,
                                             bias=cbias[:, mc, b:b + 1])
                        u_list.append(u_bf)
                    if b < NB - 1:
                        # fold v into u on the DVE: wc = v0*u0 + v1*u1
                        w0 = wpool.tile([128, S], bf16, tag="w0")
                        wc = wpool.tile([128, S], bf16, tag="wc")
                        nc.vector.tensor_scalar_mul(w0[:], u_list[0][:],
                                                    vmf[:, 0:1])
                        nc.vector.scalar_tensor_tensor(
                            wc[:], u_list[1][:], vmf[:, 1:2], w0[:],
                            op0=mybir.AluOpType.mult, op1=mybir.AluOpType.add)
                    else:
                        # last batch: keep the DVE out of the tail's
                        # dependency chain
                        wc = None
                    # score matmuls run 2 batches behind the mains; the
                    # last couple drop to lag 1 so their exps land ahead
                    # of the final tanh in the ScalarE FIFO
                    pending.append((b, wc, u_list))
                    while len(pending) >= (3 if b < NB - 2 else 2):
                        score_one()
                b0 += gsz
            while pending:
                score_one()
            while sm2:
                g, praw, sums = sm2.pop(0)
                stage2(g, praw, sums, last=(not sm2))

    nc.compile()
    return nc


def _get_nc():
    if "nc" not in _cache:
        _cache["nc"] = _build()
    return _cache["nc"]


def make_in_maps(static_hidden, dynamic_hidden, decoder_hidden, W, v):
    bf = ml_dtypes.bfloat16
    # W[0] is [H, 3H]; wt[p, kc*H + m] = W[0][m, kc*128 + p]
    wt = W[0].T.astype(bf).reshape(6, 128, H).transpose(1, 0, 2).reshape(128, 6 * H)
    # v as the moving operand: vm[p, mc] = v[mc*128 + p]
    vm = np.ascontiguousarray(v[0, 0].astype(bf).reshape(2, 128).T)

    sh = static_hidden.astype(bf).reshape(N_CORES, NB, 2, 128, S)
    dh = dynamic_hidden.astype(bf).reshape(N_CORES, NB, 2, 128, S)
    # xin[p, b, t, c, s]; h = c*128 + p
    xin_all = np.stack([sh, dh], axis=2).transpose(0, 4, 1, 2, 3, 5) \
        .reshape(N_CORES, 128, NB * 4 * S)

    in_maps = []
    for i in range(N_CORES):
        sl = slice(i * NB, (i + 1) * NB)
        dect = decoder_hidden[sl].T.astype(bf).reshape(2, 128, NB) \
            .transpose(1, 0, 2).reshape(128, 2 * NB)
        wtc = wt.reshape(128, 6, H)
        consts = np.concatenate(
            [wtc[:, 4], wtc[:, 5], dect, vm,
             wtc[:, 0], wtc[:, 1], wtc[:, 2], wtc[:, 3]], axis=1)
        in_maps.append({
            "xin": np.ascontiguousarray(xin_all[i]),
            "consts": np.ascontiguousarray(consts),
        })
    return in_maps


def kernel(static_hidden, dynamic_hidden, decoder_hidden, W, v):
    from concourse.bass_utils import run_bass_kernel_spmd

    static_hidden = np.asarray(static_hidden, dtype=np.float32)
    dynamic_hidden = np.asarray(dynamic_hidden, dtype=np.float32)
    decoder_hidden = np.asarray(decoder_hidden, dtype=np.float32)
    W = np.asarray(W, dtype=np.float32)
    v = np.asarray(v, dtype=np.float32)
    nc = _get_nc()
    in_maps = make_in_maps(static_hidden, dynamic_hidden, decoder_hidden, W, v)
    res = run_bass_kernel_spmd(nc, in_maps, list(range(N_CORES)))
    # device layout: out[p, 4b + c] = prob[b, c*128 + p]
    parts = []
    for i in range(N_CORES):
        r = res.results[i]["out"].reshape(128, NB, 4)
        parts.append(r.transpose(1, 2, 0).reshape(NB, S))
    out = np.concatenate(parts, axis=0)
    return out[:, None, :].astype(np.float32)
